# revision 2
# baseline (speedup 1.0000x reference)
"""Trainium2 kernel for nn_ClementsPSBS (Clements photonic mesh, 1024 layers).

Strategy: the whole network is linear in x (complex transfer matrix), so we
fold all 1024 layers of 2x2 rotations + attenuation into a single complex
matrix T (host-side, cheap), then the HW kernel is out = x @ T^T computed as
two real matmuls distributed over 8 NeuronCores:
  - 4 batch groups (512 rows each) x 2 column groups (real part | imag part)
  - per core: OUT[512b, 1024n] = xT[1024k, 512b]^T @ W[1024k, 1024n]
    with x-chunks stationary in the PE and W-chunks moving, fp16 in/out.
DMA: x/W chunk streams alternate between the two HWDGE queues (sync+scalar)
so input bandwidth is not bottlenecked on one descriptor ring; outputs are
evacuated per-PSUM-bank as soon as each bank's accumulation stops, so the
store overlaps the matmul tail.
"""

import numpy as np

N = 1024          # features
L = 1024          # layers
B = 2048          # batch
NA = N // 2       # pairs per layer
R_GROUPS = 4      # batch groups across cores
C_GROUPS = 2      # column groups (re | im)
BSH = B // R_GROUPS  # 512 batch rows per core

KT = N // 128     # 8 contraction chunks
BT = BSH // 128   # 4 batch tiles (PE stationary dim)
NH = N // 512     # 2 column halves (PSUM bank width)

_CACHE = {}


# ---------------------------------------------------------------------------
# Host-side fold: collapse 1024 layers into one complex transfer matrix T
# such that out = x @ T.T  (T[n, j]: coefficient of input feature j in
# output feature n).
# ---------------------------------------------------------------------------

def _expected_index():
    nA = N // 2
    iA = np.array([[2 * i, 2 * i + 1] for i in range(nA)], dtype=np.int32)
    iB = np.array([[2 * i + 1, 2 * i + 2] for i in range(nA - 1)]
                  + [[~0, ~(N - 1)]], dtype=np.int32)
    layers = [iA if l % 2 == 0 else iB for l in range(L)]
    return np.stack(layers).astype(np.int32)


def _coeffs(params, split, atten, index):
    """Per-layer per-pair 2x2 complex coefficients with attenuation folded in.

    Layer update for pair (p, q):
      u[p]' = at[p]*(cos(a)*e^{i th} * u[p] + i sin(a) * u[q])
      u[q]' = at[q]*(i sin(a)*e^{i th} * u[p] + cos(a) * u[q])
    Rows untouched by a pair still get u *= at.
    """
    theta = params[0].astype(np.float64)          # [L, NA]
    alpha = np.pi / 4 + split.astype(np.float64)  # [L, NA]
    eith = np.exp(1j * theta)
    c = np.cos(alpha)
    s = 1j * np.sin(alpha)
    A = c * eith
    Bc = s + 0j * s
    Cc = s * eith
    D = c + 0j * c
    return A, Bc, Cc, D


def _fold_fast(params, split, atten, index):
    """jax-CPU scan fold for the standard even/odd Clements pattern."""
    import jax
    import jax.numpy as jnp

    A, Bc, Cc, D = _coeffs(params, split, atten, index)
    at = atten.astype(np.complex128)              # [L, N]

    # even layers: pairs (2i, 2i+1), all N rows rotated
    ev = slice(0, L, 2)
    at_p_e = at[ev][:, 0::2]                      # [L/2, NA]
    at_q_e = at[ev][:, 1::2]
    Ae = (A[ev] * at_p_e).astype(np.complex64)
    Be = (Bc[ev] * at_p_e).astype(np.complex64)
    Ce = (Cc[ev] * at_q_e).astype(np.complex64)
    De = (D[ev] * at_q_e).astype(np.complex64)

    # odd layers: pairs (2i+1, 2i+2) for i < NA-1; rows 0 and N-1 only atten
    od = slice(1, L, 2)
    at_p_o = at[od][:, 1:N - 1:2]                 # [L/2, NA-1]
    at_q_o = at[od][:, 2:N:2]
    Ao = (A[od][:, :NA - 1] * at_p_o).astype(np.complex64)
    Bo = (Bc[od][:, :NA - 1] * at_p_o).astype(np.complex64)
    Co = (Cc[od][:, :NA - 1] * at_q_o).astype(np.complex64)
    Do = (D[od][:, :NA - 1] * at_q_o).astype(np.complex64)
    at0 = at[od][:, 0].astype(np.complex64)       # [L/2]
    atN = at[od][:, N - 1].astype(np.complex64)

    cpu = jax.devices('cpu')[0]

    def step(T, co):
        ae, be, ce, de, ao, bo, co_, do, a0, aN = co
        Tr = T.reshape(NA, 2, N)
        p = Tr[:, 0, :]
        q = Tr[:, 1, :]
        np_ = ae[:, None] * p + be[:, None] * q
        nq = ce[:, None] * p + de[:, None] * q
        T = jnp.stack([np_, nq], axis=1).reshape(N, N)
        mid = T[1:N - 1].reshape(NA - 1, 2, N)
        p = mid[:, 0, :]
        q = mid[:, 1, :]
        np_ = ao[:, None] * p + bo[:, None] * q
        nq = co_[:, None] * p + do[:, None] * q
        midn = jnp.stack([np_, nq], axis=1).reshape(N - 2, N)
        T = jnp.concatenate([T[0:1] * a0, midn, T[N - 1:] * aN], axis=0)
        return T, None

    with jax.default_device(cpu):
        T0 = jnp.eye(N, dtype=jnp.complex64)
        coeffs = (Ae, Be, Ce, De, Ao, Bo, Co, Do, at0, atN)
        coeffs = jax.tree.map(jnp.asarray, coeffs)
        fold = jax.jit(lambda T0, co: jax.lax.scan(step, T0, co)[0])
        T = fold(T0, coeffs)
        return np.asarray(T)


def _fold_general(params, split, atten, index):
    """Reference-faithful fold for arbitrary index content (numpy)."""
    A, Bc, Cc, D = _coeffs(params, split, atten, index)
    T = np.eye(N, dtype=np.complex128)
    at = atten.astype(np.complex128)
    for l in range(L):
        idx = index[l]
        valid = (idx >= 0).all(axis=1)
        gi = np.mod(idx, N)
        p = gi[valid, 0]
        q = gi[valid, 1]
        Tp = T[p, :].copy()
        Tq = T[q, :].copy()
        T[p, :] = A[l][valid][:, None] * Tp + Bc[l][valid][:, None] * Tq
        T[q, :] = Cc[l][valid][:, None] * Tp + D[l][valid][:, None] * Tq
        T *= at[l][:, None]
    return T.astype(np.complex64)


def _fold(params, split, atten, index):
    if np.array_equal(index, _expected_index()):
        try:
            return _fold_fast(params, split, atten, index)
        except Exception:
            pass
    return _fold_general(params, split, atten, index)


# ---------------------------------------------------------------------------
# Device kernel: OUT[512b, 1024n] = xT[1024k, 512b]^T @ W[1024k, 1024n]
# ---------------------------------------------------------------------------

def _build_nc():
    import concourse.bass as bass
    import concourse.bacc as bacc
    import concourse.mybir as mybir
    import concourse.tile as tile
    from contextlib import ExitStack

    f32 = mybir.dt.float32
    f16 = mybir.dt.float16

    nc = bacc.Bacc("TRN2", target_bir_lowering=False, debug=False,
                   num_devices=8)
    X = nc.dram_tensor("X", [N, BSH], f16, kind="ExternalInput").ap()
    W = nc.dram_tensor("W", [N, N], f16, kind="ExternalInput").ap()
    OUT = nc.dram_tensor("OUT", [BSH, N], f16, kind="ExternalOutput").ap()

    with tile.TileContext(nc) as tc, ExitStack() as ctx:
        xpool = ctx.enter_context(tc.tile_pool(name="xp", bufs=1))
        wpool = ctx.enter_context(tc.tile_pool(name="wp", bufs=1))
        opool = ctx.enter_context(tc.tile_pool(name="op", bufs=1))
        ppool = ctx.enter_context(tc.tile_pool(name="pp", bufs=1, space="PSUM"))

        # input streams alternate between the two HWDGE queues so neither
        # descriptor ring is the bandwidth cap and chunk k lands ~in order
        xts, wts = [], []
        for k in range(KT):
            xt = xpool.tile([128, BSH], f16, tag=f"x{k}", name=f"x{k}")
            wt = wpool.tile([128, N], f16, tag=f"w{k}", name=f"w{k}")
            e1, e2 = (nc.sync, nc.scalar) if k % 2 == 0 else (nc.scalar, nc.sync)
            e1.dma_start(out=xt[:], in_=X[128 * k:128 * (k + 1), :])
            e2.dma_start(out=wt[:], in_=W[128 * k:128 * (k + 1), :])
            xts.append(xt)
            wts.append(wt)

        # one PSUM tensor spanning all 8 banks: bank (bt, nh) holds
        # out[128*bt:128*(bt+1), 512*nh:512*(nh+1)]
        ps = ppool.tile([128, BT * NH * 512], f32, name="ps")

        # k-outer accumulation: bank (bt, nh) finishes its k=KT-1 matmul
        # (stop=True) several matmul slots before the last bank, so its
        # evacuation overlaps the matmul tail
        for k in range(KT):
            for bt in range(BT):
                for nh in range(NH):
                    nc.tensor.matmul(
                        ps[:, (bt * NH + nh) * 512:(bt * NH + nh + 1) * 512],
                        xts[k][:, 128 * bt:128 * (bt + 1)],
                        wts[k][:, 512 * nh:512 * (nh + 1)],
                        start=(k == 0),
                        stop=(k == KT - 1),
                    )

        # per-batch-tile evacuation (DVE converts f32 psum -> f16 sbuf),
        # then one output DMA per tile, alternating HWDGE queues
        for bt in range(BT):
            ot = opool.tile([128, N], f16, name=f"o{bt}")
            for nh in range(NH):
                src = ps[:, (bt * NH + nh) * 512:(bt * NH + nh + 1) * 512]
                nc.vector.tensor_copy(ot[:, 512 * nh:512 * (nh + 1)], src)
            eng = nc.sync if bt % 2 == 0 else nc.scalar
            eng.dma_start(out=OUT[128 * bt:128 * (bt + 1), :], in_=ot[:])

    nc.compile()
    return nc


def _get_nc():
    if "nc" not in _CACHE:
        _CACHE["nc"] = _build_nc()
    return _CACHE["nc"]


def _in_maps(x, T):
    """Per-core input maps: core = bg * 2 + cg (bg batch group, cg re|im)."""
    xT = np.ascontiguousarray(x.T.astype(np.float16))      # [N, B]
    Wre = np.ascontiguousarray(T.real.T.astype(np.float16))  # [j, n]
    Wim = np.ascontiguousarray(T.imag.T.astype(np.float16))
    maps = []
    for core in range(8):
        bg, cg = divmod(core, C_GROUPS)
        maps.append({
            "X": np.ascontiguousarray(xT[:, bg * BSH:(bg + 1) * BSH]),
            "W": Wre if cg == 0 else Wim,
        })
    return maps


def _assemble(results):
    out = np.empty((B, N), dtype=np.complex64)
    for core in range(8):
        bg, cg = divmod(core, C_GROUPS)
        o = results[core]["OUT"].astype(np.float32)        # [BSH, N]
        if cg == 0:
            out.real[bg * BSH:(bg + 1) * BSH, :] = o
        else:
            out.imag[bg * BSH:(bg + 1) * BSH, :] = o
    return out


def kernel(x, params, split, atten, index):
    from concourse.bass_utils import run_bass_kernel_spmd

    x = np.asarray(x, dtype=np.float32)
    T = _fold(np.asarray(params), np.asarray(split), np.asarray(atten),
              np.asarray(index))
    nc = _get_nc()
    res = run_bass_kernel_spmd(nc, _in_maps(x, T), list(range(8)))
    return _assemble(res.results)


# revision 4
# speedup vs baseline: 1.0347x; 1.0347x over previous
"""Trainium2 kernel for nn_ClementsPSBS (Clements photonic mesh, 1024 layers).

Strategy: the whole network is linear in x (complex transfer matrix), so we
fold all 1024 layers of 2x2 rotations + attenuation into a single complex
matrix T (host-side, cheap), then the HW kernel is out = x @ T^T computed as
two real matmuls distributed over 8 NeuronCores:
  - 4 batch groups (512 rows each) x 2 column groups (real part | imag part)
  - per core: OUT[512b, 1024n] = xT[1024k, 512b]^T @ W[1024k, 1024n]
    with x-chunks stationary in the PE and W-chunks moving, fp16 in/out.
DMA: x/W chunk streams alternate between the two HWDGE queues (sync+scalar)
so input bandwidth is not bottlenecked on one descriptor ring; outputs are
evacuated per-PSUM-bank as soon as each bank's accumulation stops, so the
store overlaps the matmul tail.
"""

import numpy as np

N = 1024          # features
L = 1024          # layers
B = 2048          # batch
NA = N // 2       # pairs per layer
R_GROUPS = 4      # batch groups across cores
C_GROUPS = 2      # column groups (re | im)
BSH = B // R_GROUPS  # 512 batch rows per core

KT = N // 128     # 8 contraction chunks
BT = BSH // 128   # 4 batch tiles (PE stationary dim)
NH = N // 512     # 2 column halves (PSUM bank width)

_CACHE = {}


# ---------------------------------------------------------------------------
# Host-side fold: collapse 1024 layers into one complex transfer matrix T
# such that out = x @ T.T  (T[n, j]: coefficient of input feature j in
# output feature n).
# ---------------------------------------------------------------------------

def _expected_index():
    nA = N // 2
    iA = np.array([[2 * i, 2 * i + 1] for i in range(nA)], dtype=np.int32)
    iB = np.array([[2 * i + 1, 2 * i + 2] for i in range(nA - 1)]
                  + [[~0, ~(N - 1)]], dtype=np.int32)
    layers = [iA if l % 2 == 0 else iB for l in range(L)]
    return np.stack(layers).astype(np.int32)


def _coeffs(params, split, atten, index):
    """Per-layer per-pair 2x2 complex coefficients with attenuation folded in.

    Layer update for pair (p, q):
      u[p]' = at[p]*(cos(a)*e^{i th} * u[p] + i sin(a) * u[q])
      u[q]' = at[q]*(i sin(a)*e^{i th} * u[p] + cos(a) * u[q])
    Rows untouched by a pair still get u *= at.
    """
    theta = params[0].astype(np.float64)          # [L, NA]
    alpha = np.pi / 4 + split.astype(np.float64)  # [L, NA]
    eith = np.exp(1j * theta)
    c = np.cos(alpha)
    s = 1j * np.sin(alpha)
    A = c * eith
    Bc = s + 0j * s
    Cc = s * eith
    D = c + 0j * c
    return A, Bc, Cc, D


def _fold_fast(params, split, atten, index):
    """jax-CPU scan fold for the standard even/odd Clements pattern."""
    import jax
    import jax.numpy as jnp

    A, Bc, Cc, D = _coeffs(params, split, atten, index)
    at = atten.astype(np.complex128)              # [L, N]

    # even layers: pairs (2i, 2i+1), all N rows rotated
    ev = slice(0, L, 2)
    at_p_e = at[ev][:, 0::2]                      # [L/2, NA]
    at_q_e = at[ev][:, 1::2]
    Ae = (A[ev] * at_p_e).astype(np.complex64)
    Be = (Bc[ev] * at_p_e).astype(np.complex64)
    Ce = (Cc[ev] * at_q_e).astype(np.complex64)
    De = (D[ev] * at_q_e).astype(np.complex64)

    # odd layers: pairs (2i+1, 2i+2) for i < NA-1; rows 0 and N-1 only atten
    od = slice(1, L, 2)
    at_p_o = at[od][:, 1:N - 1:2]                 # [L/2, NA-1]
    at_q_o = at[od][:, 2:N:2]
    Ao = (A[od][:, :NA - 1] * at_p_o).astype(np.complex64)
    Bo = (Bc[od][:, :NA - 1] * at_p_o).astype(np.complex64)
    Co = (Cc[od][:, :NA - 1] * at_q_o).astype(np.complex64)
    Do = (D[od][:, :NA - 1] * at_q_o).astype(np.complex64)
    at0 = at[od][:, 0].astype(np.complex64)       # [L/2]
    atN = at[od][:, N - 1].astype(np.complex64)

    cpu = jax.devices('cpu')[0]

    def step(T, co):
        ae, be, ce, de, ao, bo, co_, do, a0, aN = co
        Tr = T.reshape(NA, 2, N)
        p = Tr[:, 0, :]
        q = Tr[:, 1, :]
        np_ = ae[:, None] * p + be[:, None] * q
        nq = ce[:, None] * p + de[:, None] * q
        T = jnp.stack([np_, nq], axis=1).reshape(N, N)
        mid = T[1:N - 1].reshape(NA - 1, 2, N)
        p = mid[:, 0, :]
        q = mid[:, 1, :]
        np_ = ao[:, None] * p + bo[:, None] * q
        nq = co_[:, None] * p + do[:, None] * q
        midn = jnp.stack([np_, nq], axis=1).reshape(N - 2, N)
        T = jnp.concatenate([T[0:1] * a0, midn, T[N - 1:] * aN], axis=0)
        return T, None

    with jax.default_device(cpu):
        T0 = jnp.eye(N, dtype=jnp.complex64)
        coeffs = (Ae, Be, Ce, De, Ao, Bo, Co, Do, at0, atN)
        coeffs = jax.tree.map(jnp.asarray, coeffs)
        fold = jax.jit(lambda T0, co: jax.lax.scan(step, T0, co)[0])
        T = fold(T0, coeffs)
        return np.asarray(T)


def _fold_general(params, split, atten, index):
    """Reference-faithful fold for arbitrary index content (numpy)."""
    A, Bc, Cc, D = _coeffs(params, split, atten, index)
    T = np.eye(N, dtype=np.complex128)
    at = atten.astype(np.complex128)
    for l in range(L):
        idx = index[l]
        valid = (idx >= 0).all(axis=1)
        gi = np.mod(idx, N)
        p = gi[valid, 0]
        q = gi[valid, 1]
        Tp = T[p, :].copy()
        Tq = T[q, :].copy()
        T[p, :] = A[l][valid][:, None] * Tp + Bc[l][valid][:, None] * Tq
        T[q, :] = Cc[l][valid][:, None] * Tp + D[l][valid][:, None] * Tq
        T *= at[l][:, None]
    return T.astype(np.complex64)


def _fold(params, split, atten, index):
    if np.array_equal(index, _expected_index()):
        try:
            return _fold_fast(params, split, atten, index)
        except Exception:
            pass
    return _fold_general(params, split, atten, index)


# ---------------------------------------------------------------------------
# Device kernel: OUT[512b, 1024n] = xT[1024k, 512b]^T @ W[1024k, 1024n]
# ---------------------------------------------------------------------------

N_WARMUP = 16     # PE p-state warmup matmuls before real data arrives


def _build_nc():
    import concourse.bass as bass
    import concourse.bacc as bacc
    import concourse.mybir as mybir
    import concourse.tile as tile
    from contextlib import ExitStack

    f32 = mybir.dt.float32
    f16 = mybir.dt.float16

    nc = bacc.Bacc("TRN2", target_bir_lowering=False, debug=False,
                   num_devices=8)
    # X2: x-shard repacked so each 128-partition tile holds TWO k-chunks
    # (2048B contiguous per partition row -> full-size DMA descriptors):
    # X2[kc*128 + p, i*512 + b] = xT[kc*256 + i*128 + p, b]
    X2 = nc.dram_tensor("X2", [N // 2, 2 * BSH], f16, kind="ExternalInput").ap()
    W = nc.dram_tensor("W", [N, N], f16, kind="ExternalInput").ap()
    OUT = nc.dram_tensor("OUT", [BSH, N], f16, kind="ExternalOutput").ap()

    with tile.TileContext(nc) as tc, ExitStack() as ctx:
        xpool = ctx.enter_context(tc.tile_pool(name="xp", bufs=1))
        wpool = ctx.enter_context(tc.tile_pool(name="wp", bufs=1))
        opool = ctx.enter_context(tc.tile_pool(name="op", bufs=1))
        ppool = ctx.enter_context(tc.tile_pool(name="pp", bufs=1, space="PSUM"))

        # interleave the two HWDGE queues (sync, scalar) so chunk k data
        # lands roughly in consumption order and both rings share the load
        xts = [xpool.tile([128, 2 * BSH], f16, tag=f"x{c}", name=f"x{c}")
               for c in range(KT // 2)]
        wts = [wpool.tile([128, N], f16, tag=f"w{k}", name=f"w{k}")
               for k in range(KT)]

        def ldx(c, eng):
            eng.dma_start(out=xts[c][:], in_=X2[128 * c:128 * (c + 1), :])

        def ldw(k, eng):
            eng.dma_start(out=wts[k][:], in_=W[128 * k:128 * (k + 1), :])

        ldx(0, nc.sync)
        ldw(0, nc.scalar)
        ldw(1, nc.sync)
        ldw(2, nc.scalar)
        ldx(1, nc.sync)
        ldw(3, nc.scalar)
        ldw(4, nc.sync)
        ldx(2, nc.scalar)
        ldw(5, nc.sync)
        ldw(6, nc.scalar)
        ldx(3, nc.sync)
        ldw(7, nc.scalar)

        # PE p-state warmup: the PE clock ramps 0.65 -> 1.2 -> 2.4 GHz only
        # after ~3us of continuous execution, so burn the ramp on dummy
        # matmuls over a memset tile while the first input chunks stream in.
        wa = opool.tile([128, 512], f16, name="warm")
        nc.gpsimd.memset(wa[:], 0.0)
        ps = ppool.tile([128, BT * NH * 512], f32, name="ps")
        for i in range(N_WARMUP):
            nc.tensor.matmul(
                ps[:, 0:512], wa[:, 0:128], wa[:],
                start=True, stop=True, skip_group_check=True,
            )

        # bank (bt, nh) holds out[128*bt:128*(bt+1), 512*nh:512*(nh+1)];
        # bank-outer program order: the Tile scheduler re-pipelines it into
        # chunk-paced order, and each bank's k=KT-1 stop lands as soon as
        # the last chunk + leftover work allows, staggering the evacuations
        for bt in range(BT):
            for nh in range(NH):
                for k in range(KT):
                    nc.tensor.matmul(
                        ps[:, (bt * NH + nh) * 512:(bt * NH + nh + 1) * 512],
                        xts[k // 2][:, (k % 2) * BSH + 128 * bt:
                                    (k % 2) * BSH + 128 * (bt + 1)],
                        wts[k][:, 512 * nh:512 * (nh + 1)],
                        start=(k == 0),
                        stop=(k == KT - 1),
                    )
            # evacuate this batch-tile as soon as both banks stopped:
            # DVE + ACT in parallel (both convert f32 psum -> f16 sbuf)
            ot = opool.tile([128, N], f16, name=f"o{bt}")
            nc.vector.tensor_copy(
                ot[:, 0:512], ps[:, (bt * NH) * 512:(bt * NH) * 512 + 512])
            nc.scalar.copy(
                ot[:, 512:1024],
                ps[:, (bt * NH + 1) * 512:(bt * NH + 1) * 512 + 512])
            eng = nc.sync if bt % 2 == 0 else nc.scalar
            eng.dma_start(out=OUT[128 * bt:128 * (bt + 1), :], in_=ot[:])

    nc.compile()
    return nc


def _get_nc():
    if "nc" not in _CACHE:
        _CACHE["nc"] = _build_nc()
    return _CACHE["nc"]


def _in_maps(x, T):
    """Per-core input maps: core = bg * 2 + cg (bg batch group, cg re|im)."""
    xT = x.T.astype(np.float16)                            # [N, B]
    Wre = np.ascontiguousarray(T.real.T.astype(np.float16))  # [j, n]
    Wim = np.ascontiguousarray(T.imag.T.astype(np.float16))
    maps = []
    for core in range(8):
        bg, cg = divmod(core, C_GROUPS)
        xs = xT[:, bg * BSH:(bg + 1) * BSH]                # [N, BSH]
        # X2[kc*128 + p, i*512 + b] = xs[kc*256 + i*128 + p, b]
        x2 = np.ascontiguousarray(
            xs.reshape(N // 256, 2, 128, BSH)
            .transpose(0, 2, 1, 3)
            .reshape(N // 2, 2 * BSH))
        maps.append({
            "X2": x2,
            "W": Wre if cg == 0 else Wim,
        })
    return maps


def _assemble(results):
    out = np.empty((B, N), dtype=np.complex64)
    for core in range(8):
        bg, cg = divmod(core, C_GROUPS)
        o = results[core]["OUT"].astype(np.float32)        # [BSH, N]
        if cg == 0:
            out.real[bg * BSH:(bg + 1) * BSH, :] = o
        else:
            out.imag[bg * BSH:(bg + 1) * BSH, :] = o
    return out


def kernel(x, params, split, atten, index):
    from concourse.bass_utils import run_bass_kernel_spmd

    x = np.asarray(x, dtype=np.float32)
    T = _fold(np.asarray(params), np.asarray(split), np.asarray(atten),
              np.asarray(index))
    nc = _get_nc()
    res = run_bass_kernel_spmd(nc, _in_maps(x, T), list(range(8)))
    return _assemble(res.results)


# revision 6
# speedup vs baseline: 1.0553x; 1.0199x over previous
"""Trainium2 kernel for nn_ClementsPSBS (Clements photonic mesh, 1024 layers).

Strategy: the whole network is linear in x (complex transfer matrix), so we
fold all 1024 layers of 2x2 rotations + attenuation into a single complex
matrix T (host-side, cheap), then the HW kernel is out = x @ T^T computed as
two real matmuls distributed over 8 NeuronCores:
  - 4 batch groups (512 rows each) x 2 column groups (real part | imag part)
  - per core: OUT[512b, 1024n] = xT[1024k, 512b]^T @ W[1024k, 1024n]
    with x-chunks stationary in the PE and W-chunks moving, fp16 in/out.
DMA: x/W chunk streams alternate between the two HWDGE queues (sync+scalar)
so input bandwidth is not bottlenecked on one descriptor ring; outputs are
evacuated per-PSUM-bank as soon as each bank's accumulation stops, so the
store overlaps the matmul tail.
"""

import numpy as np

N = 1024          # features
L = 1024          # layers
B = 2048          # batch
NA = N // 2       # pairs per layer
R_GROUPS = 4      # batch groups across cores
C_GROUPS = 2      # column groups (re | im)
BSH = B // R_GROUPS  # 512 batch rows per core

KT = N // 128     # 8 contraction chunks
BT = BSH // 128   # 4 batch tiles (PE stationary dim)
NH = N // 512     # 2 column halves (PSUM bank width)

_CACHE = {}


# ---------------------------------------------------------------------------
# Host-side fold: collapse 1024 layers into one complex transfer matrix T
# such that out = x @ T.T  (T[n, j]: coefficient of input feature j in
# output feature n).
# ---------------------------------------------------------------------------

def _expected_index():
    nA = N // 2
    iA = np.array([[2 * i, 2 * i + 1] for i in range(nA)], dtype=np.int32)
    iB = np.array([[2 * i + 1, 2 * i + 2] for i in range(nA - 1)]
                  + [[~0, ~(N - 1)]], dtype=np.int32)
    layers = [iA if l % 2 == 0 else iB for l in range(L)]
    return np.stack(layers).astype(np.int32)


def _coeffs(params, split, atten, index):
    """Per-layer per-pair 2x2 complex coefficients with attenuation folded in.

    Layer update for pair (p, q):
      u[p]' = at[p]*(cos(a)*e^{i th} * u[p] + i sin(a) * u[q])
      u[q]' = at[q]*(i sin(a)*e^{i th} * u[p] + cos(a) * u[q])
    Rows untouched by a pair still get u *= at.
    """
    theta = params[0].astype(np.float64)          # [L, NA]
    alpha = np.pi / 4 + split.astype(np.float64)  # [L, NA]
    eith = np.exp(1j * theta)
    c = np.cos(alpha)
    s = 1j * np.sin(alpha)
    A = c * eith
    Bc = s + 0j * s
    Cc = s * eith
    D = c + 0j * c
    return A, Bc, Cc, D


def _fold_fast(params, split, atten, index):
    """jax-CPU scan fold for the standard even/odd Clements pattern."""
    import jax
    import jax.numpy as jnp

    A, Bc, Cc, D = _coeffs(params, split, atten, index)
    at = atten.astype(np.complex128)              # [L, N]

    # even layers: pairs (2i, 2i+1), all N rows rotated
    ev = slice(0, L, 2)
    at_p_e = at[ev][:, 0::2]                      # [L/2, NA]
    at_q_e = at[ev][:, 1::2]
    Ae = (A[ev] * at_p_e).astype(np.complex64)
    Be = (Bc[ev] * at_p_e).astype(np.complex64)
    Ce = (Cc[ev] * at_q_e).astype(np.complex64)
    De = (D[ev] * at_q_e).astype(np.complex64)

    # odd layers: pairs (2i+1, 2i+2) for i < NA-1; rows 0 and N-1 only atten
    od = slice(1, L, 2)
    at_p_o = at[od][:, 1:N - 1:2]                 # [L/2, NA-1]
    at_q_o = at[od][:, 2:N:2]
    Ao = (A[od][:, :NA - 1] * at_p_o).astype(np.complex64)
    Bo = (Bc[od][:, :NA - 1] * at_p_o).astype(np.complex64)
    Co = (Cc[od][:, :NA - 1] * at_q_o).astype(np.complex64)
    Do = (D[od][:, :NA - 1] * at_q_o).astype(np.complex64)
    at0 = at[od][:, 0].astype(np.complex64)       # [L/2]
    atN = at[od][:, N - 1].astype(np.complex64)

    cpu = jax.devices('cpu')[0]

    def step(T, co):
        ae, be, ce, de, ao, bo, co_, do, a0, aN = co
        Tr = T.reshape(NA, 2, N)
        p = Tr[:, 0, :]
        q = Tr[:, 1, :]
        np_ = ae[:, None] * p + be[:, None] * q
        nq = ce[:, None] * p + de[:, None] * q
        T = jnp.stack([np_, nq], axis=1).reshape(N, N)
        mid = T[1:N - 1].reshape(NA - 1, 2, N)
        p = mid[:, 0, :]
        q = mid[:, 1, :]
        np_ = ao[:, None] * p + bo[:, None] * q
        nq = co_[:, None] * p + do[:, None] * q
        midn = jnp.stack([np_, nq], axis=1).reshape(N - 2, N)
        T = jnp.concatenate([T[0:1] * a0, midn, T[N - 1:] * aN], axis=0)
        return T, None

    with jax.default_device(cpu):
        T0 = jnp.eye(N, dtype=jnp.complex64)
        coeffs = (Ae, Be, Ce, De, Ao, Bo, Co, Do, at0, atN)
        coeffs = jax.tree.map(jnp.asarray, coeffs)
        fold = jax.jit(lambda T0, co: jax.lax.scan(step, T0, co)[0])
        T = fold(T0, coeffs)
        return np.asarray(T)


def _fold_general(params, split, atten, index):
    """Reference-faithful fold for arbitrary index content (numpy)."""
    A, Bc, Cc, D = _coeffs(params, split, atten, index)
    T = np.eye(N, dtype=np.complex128)
    at = atten.astype(np.complex128)
    for l in range(L):
        idx = index[l]
        valid = (idx >= 0).all(axis=1)
        gi = np.mod(idx, N)
        p = gi[valid, 0]
        q = gi[valid, 1]
        Tp = T[p, :].copy()
        Tq = T[q, :].copy()
        T[p, :] = A[l][valid][:, None] * Tp + Bc[l][valid][:, None] * Tq
        T[q, :] = Cc[l][valid][:, None] * Tp + D[l][valid][:, None] * Tq
        T *= at[l][:, None]
    return T.astype(np.complex64)


def _fold(params, split, atten, index):
    if np.array_equal(index, _expected_index()):
        try:
            return _fold_fast(params, split, atten, index)
        except Exception:
            pass
    return _fold_general(params, split, atten, index)


# ---------------------------------------------------------------------------
# Device kernel: OUT[512b, 1024n] = xT[1024k, 512b]^T @ W[1024k, 1024n]
# ---------------------------------------------------------------------------

N_WARMUP = 5      # PE p-state warmup matmuls before real data arrives


def _build_nc():
    import concourse.bass as bass
    import concourse.bacc as bacc
    import concourse.mybir as mybir
    import concourse.tile as tile
    from contextlib import ExitStack

    f32 = mybir.dt.float32
    f16 = mybir.dt.float16

    nc = bacc.Bacc("TRN2", target_bir_lowering=False, debug=False,
                   num_devices=8)
    # X2: x-shard repacked so each 128-partition tile holds TWO k-chunks
    # (2048B contiguous per partition row -> full-size DMA descriptors):
    # X2[kc*128 + p, i*512 + b] = xT[kc*256 + i*128 + p, b]
    X2 = nc.dram_tensor("X2", [N // 2, 2 * BSH], f16, kind="ExternalInput").ap()
    W = nc.dram_tensor("W", [N, N], f16, kind="ExternalInput").ap()
    OUT = nc.dram_tensor("OUT", [BSH, N], f16, kind="ExternalOutput").ap()

    with tile.TileContext(nc) as tc, ExitStack() as ctx:
        xpool = ctx.enter_context(tc.tile_pool(name="xp", bufs=1))
        wpool = ctx.enter_context(tc.tile_pool(name="wp", bufs=1))
        opool = ctx.enter_context(tc.tile_pool(name="op", bufs=1))
        ppool = ctx.enter_context(tc.tile_pool(name="pp", bufs=1, space="PSUM"))

        # Input tiles. The first pair of k-chunks is split into standalone
        # half-size tiles so the very first matmul only waits on 128KB per
        # queue; later chunks stay packed (2048B descriptors).
        x0a = xpool.tile([128, BSH], f16, tag="x0a", name="x0a")   # k=0
        x0b = xpool.tile([128, BSH], f16, tag="x0b", name="x0b")   # k=1
        xts = [xpool.tile([128, 2 * BSH], f16, tag=f"x{c}", name=f"x{c}")
               for c in range(1, KT // 2)]
        wts = [wpool.tile([128, N], f16, tag=f"w{k}", name=f"w{k}")
               for k in range(KT)]

        def xsrc(k, bt):
            """lhsT access for contraction chunk k, batch tile bt."""
            if k == 0:
                return x0a[:, 128 * bt:128 * (bt + 1)]
            if k == 1:
                return x0b[:, 128 * bt:128 * (bt + 1)]
            c = k // 2 - 1
            off = (k % 2) * BSH
            return xts[c][:, off + 128 * bt:off + 128 * (bt + 1)]

        # issue order = chunk consumption order, alternating HWDGE queues
        # (sync/scalar) so both descriptor rings share the load and chunk k
        # lands roughly in order
        issue = [
            ("x", x0a, (0, 0, 1)),            # X2 row block 0, first half
            ("w", wts[0], None),
            ("x", x0b, (0, 1, 2)),            # X2 row block 0, second half
            ("w", wts[1], None),
            ("w", wts[2], None),
            ("x", xts[0], (1, 0, 2)),         # k=2,3
            ("w", wts[3], None),
            ("w", wts[4], None),
            ("x", xts[1], (2, 0, 2)),         # k=4,5
            ("w", wts[5], None),
            ("w", wts[6], None),
            ("x", xts[2], (3, 0, 2)),         # k=6,7
            ("w", wts[7], None),
        ]
        wk = 0
        for j, (kind, t, meta) in enumerate(issue):
            eng = nc.sync if j % 2 == 0 else nc.scalar
            if kind == "x":
                c, h0, h1 = meta
                eng.dma_start(
                    out=t[:],
                    in_=X2[128 * c:128 * (c + 1), h0 * BSH:h1 * BSH])
            else:
                eng.dma_start(out=t[:], in_=W[128 * wk:128 * (wk + 1), :])
                wk += 1

        # PE p-state warmup: the PE clock ramps 0.65 -> 1.2 -> 2.4 GHz only
        # after ~3us of continuous execution; burn part of the ramp on dummy
        # matmuls over a memset tile while the first chunks stream in.
        wa = opool.tile([128, 512], f16, name="warm")
        nc.gpsimd.memset(wa[:], 0.0)
        ps = ppool.tile([128, BT * NH * 512], f32, name="ps")
        for i in range(N_WARMUP):
            nc.tensor.matmul(
                ps[:, 0:512], wa[:, 0:128], wa[:],
                start=True, stop=True, skip_group_check=True,
            )

        # bank (bt, nh) holds out[128*bt:128*(bt+1), 512*nh:512*(nh+1)];
        # bank-outer program order: the Tile scheduler re-pipelines it into
        # chunk-paced order, and each bank's k=KT-1 stop lands as soon as
        # the last chunk + leftover work allows, staggering the evacuations
        ots = [opool.tile([128, N], f16, name=f"o{bt}") for bt in range(BT)]
        for bt in range(BT):
            for nh in range(NH):
                bank = ps[:, (bt * NH + nh) * 512:(bt * NH + nh + 1) * 512]
                for k in range(KT):
                    nc.tensor.matmul(
                        bank, xsrc(k, bt), wts[k][:, 512 * nh:512 * (nh + 1)],
                        start=(k == 0),
                        stop=(k == KT - 1),
                    )
                # evacuate this bank as soon as it stops (DVE converts
                # f32 psum -> f16 sbuf; banks stop ~8 matmul slots apart so
                # a single DVE keeps up), then store it immediately
                ot = ots[bt]
                nc.vector.tensor_copy(ot[:, 512 * nh:512 * (nh + 1)], bank)
                eng = nc.sync if (bt * NH + nh) % 2 == 0 else nc.scalar
                eng.dma_start(
                    out=OUT[128 * bt:128 * (bt + 1),
                            512 * nh:512 * (nh + 1)],
                    in_=ot[:, 512 * nh:512 * (nh + 1)])

    nc.compile()
    return nc


def _get_nc():
    if "nc" not in _CACHE:
        _CACHE["nc"] = _build_nc()
    return _CACHE["nc"]


def _in_maps(x, T):
    """Per-core input maps: core = bg * 2 + cg (bg batch group, cg re|im)."""
    xT = x.T.astype(np.float16)                            # [N, B]
    Wre = np.ascontiguousarray(T.real.T.astype(np.float16))  # [j, n]
    Wim = np.ascontiguousarray(T.imag.T.astype(np.float16))
    maps = []
    for core in range(8):
        bg, cg = divmod(core, C_GROUPS)
        xs = xT[:, bg * BSH:(bg + 1) * BSH]                # [N, BSH]
        # X2[kc*128 + p, i*512 + b] = xs[kc*256 + i*128 + p, b]
        x2 = np.ascontiguousarray(
            xs.reshape(N // 256, 2, 128, BSH)
            .transpose(0, 2, 1, 3)
            .reshape(N // 2, 2 * BSH))
        maps.append({
            "X2": x2,
            "W": Wre if cg == 0 else Wim,
        })
    return maps


def _assemble(results):
    out = np.empty((B, N), dtype=np.complex64)
    for core in range(8):
        bg, cg = divmod(core, C_GROUPS)
        o = results[core]["OUT"].astype(np.float32)        # [BSH, N]
        if cg == 0:
            out.real[bg * BSH:(bg + 1) * BSH, :] = o
        else:
            out.imag[bg * BSH:(bg + 1) * BSH, :] = o
    return out


def kernel(x, params, split, atten, index):
    from concourse.bass_utils import run_bass_kernel_spmd

    x = np.asarray(x, dtype=np.float32)
    T = _fold(np.asarray(params), np.asarray(split), np.asarray(atten),
              np.asarray(index))
    nc = _get_nc()
    res = run_bass_kernel_spmd(nc, _in_maps(x, T), list(range(8)))
    return _assemble(res.results)


# revision 7
# speedup vs baseline: 1.0998x; 1.0422x over previous
"""Trainium2 kernel for nn_ClementsPSBS (Clements photonic mesh, 1024 layers).

Strategy: the whole network is linear in x (complex transfer matrix), so we
fold all 1024 layers of 2x2 rotations + attenuation into a single complex
matrix T (host-side, cheap), then the HW kernel is out = x @ T^T computed as
two real matmuls distributed over 8 NeuronCores:
  - 4 batch groups (512 rows each) x 2 column groups (real part | imag part)
  - per core: OUT[512b, 1024n] = xT[1024k, 512b]^T @ W[1024k, 1024n]
    with x-chunks stationary in the PE and W-chunks moving, fp16 in/out.
DMA: x/W chunk streams alternate between the two HWDGE queues (sync+scalar)
so input bandwidth is not bottlenecked on one descriptor ring; outputs are
evacuated per-PSUM-bank as soon as each bank's accumulation stops, so the
store overlaps the matmul tail.
"""

import numpy as np

N = 1024          # features
L = 1024          # layers
B = 2048          # batch
NA = N // 2       # pairs per layer
R_GROUPS = 4      # batch groups across cores
C_GROUPS = 2      # column groups (re | im)
BSH = B // R_GROUPS  # 512 batch rows per core

KT = N // 128     # 8 contraction chunks
BT = BSH // 128   # 4 batch tiles (PE stationary dim)
NH = N // 512     # 2 column halves (PSUM bank width)

_CACHE = {}


# ---------------------------------------------------------------------------
# Host-side fold: collapse 1024 layers into one complex transfer matrix T
# such that out = x @ T.T  (T[n, j]: coefficient of input feature j in
# output feature n).
# ---------------------------------------------------------------------------

def _expected_index():
    nA = N // 2
    iA = np.array([[2 * i, 2 * i + 1] for i in range(nA)], dtype=np.int32)
    iB = np.array([[2 * i + 1, 2 * i + 2] for i in range(nA - 1)]
                  + [[~0, ~(N - 1)]], dtype=np.int32)
    layers = [iA if l % 2 == 0 else iB for l in range(L)]
    return np.stack(layers).astype(np.int32)


def _coeffs(params, split, atten, index):
    """Per-layer per-pair 2x2 complex coefficients with attenuation folded in.

    Layer update for pair (p, q):
      u[p]' = at[p]*(cos(a)*e^{i th} * u[p] + i sin(a) * u[q])
      u[q]' = at[q]*(i sin(a)*e^{i th} * u[p] + cos(a) * u[q])
    Rows untouched by a pair still get u *= at.
    """
    theta = params[0].astype(np.float64)          # [L, NA]
    alpha = np.pi / 4 + split.astype(np.float64)  # [L, NA]
    eith = np.exp(1j * theta)
    c = np.cos(alpha)
    s = 1j * np.sin(alpha)
    A = c * eith
    Bc = s + 0j * s
    Cc = s * eith
    D = c + 0j * c
    return A, Bc, Cc, D


def _fold_fast(params, split, atten, index):
    """jax-CPU scan fold for the standard even/odd Clements pattern."""
    import jax
    import jax.numpy as jnp

    A, Bc, Cc, D = _coeffs(params, split, atten, index)
    at = atten.astype(np.complex128)              # [L, N]

    # even layers: pairs (2i, 2i+1), all N rows rotated
    ev = slice(0, L, 2)
    at_p_e = at[ev][:, 0::2]                      # [L/2, NA]
    at_q_e = at[ev][:, 1::2]
    Ae = (A[ev] * at_p_e).astype(np.complex64)
    Be = (Bc[ev] * at_p_e).astype(np.complex64)
    Ce = (Cc[ev] * at_q_e).astype(np.complex64)
    De = (D[ev] * at_q_e).astype(np.complex64)

    # odd layers: pairs (2i+1, 2i+2) for i < NA-1; rows 0 and N-1 only atten
    od = slice(1, L, 2)
    at_p_o = at[od][:, 1:N - 1:2]                 # [L/2, NA-1]
    at_q_o = at[od][:, 2:N:2]
    Ao = (A[od][:, :NA - 1] * at_p_o).astype(np.complex64)
    Bo = (Bc[od][:, :NA - 1] * at_p_o).astype(np.complex64)
    Co = (Cc[od][:, :NA - 1] * at_q_o).astype(np.complex64)
    Do = (D[od][:, :NA - 1] * at_q_o).astype(np.complex64)
    at0 = at[od][:, 0].astype(np.complex64)       # [L/2]
    atN = at[od][:, N - 1].astype(np.complex64)

    cpu = jax.devices('cpu')[0]

    def step(T, co):
        ae, be, ce, de, ao, bo, co_, do, a0, aN = co
        Tr = T.reshape(NA, 2, N)
        p = Tr[:, 0, :]
        q = Tr[:, 1, :]
        np_ = ae[:, None] * p + be[:, None] * q
        nq = ce[:, None] * p + de[:, None] * q
        T = jnp.stack([np_, nq], axis=1).reshape(N, N)
        mid = T[1:N - 1].reshape(NA - 1, 2, N)
        p = mid[:, 0, :]
        q = mid[:, 1, :]
        np_ = ao[:, None] * p + bo[:, None] * q
        nq = co_[:, None] * p + do[:, None] * q
        midn = jnp.stack([np_, nq], axis=1).reshape(N - 2, N)
        T = jnp.concatenate([T[0:1] * a0, midn, T[N - 1:] * aN], axis=0)
        return T, None

    with jax.default_device(cpu):
        T0 = jnp.eye(N, dtype=jnp.complex64)
        coeffs = (Ae, Be, Ce, De, Ao, Bo, Co, Do, at0, atN)
        coeffs = jax.tree.map(jnp.asarray, coeffs)
        fold = jax.jit(lambda T0, co: jax.lax.scan(step, T0, co)[0])
        T = fold(T0, coeffs)
        return np.asarray(T)


def _fold_general(params, split, atten, index):
    """Reference-faithful fold for arbitrary index content (numpy)."""
    A, Bc, Cc, D = _coeffs(params, split, atten, index)
    T = np.eye(N, dtype=np.complex128)
    at = atten.astype(np.complex128)
    for l in range(L):
        idx = index[l]
        valid = (idx >= 0).all(axis=1)
        gi = np.mod(idx, N)
        p = gi[valid, 0]
        q = gi[valid, 1]
        Tp = T[p, :].copy()
        Tq = T[q, :].copy()
        T[p, :] = A[l][valid][:, None] * Tp + Bc[l][valid][:, None] * Tq
        T[q, :] = Cc[l][valid][:, None] * Tp + D[l][valid][:, None] * Tq
        T *= at[l][:, None]
    return T.astype(np.complex64)


def _fold(params, split, atten, index):
    if np.array_equal(index, _expected_index()):
        try:
            return _fold_fast(params, split, atten, index)
        except Exception:
            pass
    return _fold_general(params, split, atten, index)


# ---------------------------------------------------------------------------
# Device kernel: OUT[512b, 1024n] = xT[1024k, 512b]^T @ W[1024k, 1024n]
# ---------------------------------------------------------------------------

N_WARMUP = 5      # PE p-state warmup matmuls before real data arrives


def _build_nc():
    import concourse.bass as bass
    import concourse.bacc as bacc
    import concourse.mybir as mybir
    import concourse.tile as tile
    from contextlib import ExitStack

    f32 = mybir.dt.float32
    f16 = mybir.dt.float16

    nc = bacc.Bacc("TRN2", target_bir_lowering=False, debug=False,
                   num_devices=8)
    # X2: x-shard repacked so each 128-partition tile holds TWO k-chunks
    # (2048B contiguous per partition row -> full-size DMA descriptors):
    # X2[kc*128 + p, i*512 + b] = xT[kc*256 + i*128 + p, b]
    X2 = nc.dram_tensor("X2", [N // 2, 2 * BSH], f16, kind="ExternalInput").ap()
    W = nc.dram_tensor("W", [N, N], f16, kind="ExternalInput").ap()
    OUT = nc.dram_tensor("OUT", [BSH, N], f16, kind="ExternalOutput").ap()

    with tile.TileContext(nc) as tc, ExitStack() as ctx:
        xpool = ctx.enter_context(tc.tile_pool(name="xp", bufs=1))
        wpool = ctx.enter_context(tc.tile_pool(name="wp", bufs=1))
        opool = ctx.enter_context(tc.tile_pool(name="op", bufs=1))
        ppool = ctx.enter_context(tc.tile_pool(name="pp", bufs=1, space="PSUM"))

        # Input tiles. The first k-chunk is split into half-size tiles so
        # the very first matmul only waits on 128KB per queue; later chunks
        # stay packed (2048B descriptors).
        x0a = xpool.tile([128, BSH], f16, tag="x0a", name="x0a")   # k=0
        x0b = xpool.tile([128, BSH], f16, tag="x0b", name="x0b")   # k=1
        xts = [xpool.tile([128, 2 * BSH], f16, tag=f"x{c}", name=f"x{c}")
               for c in range(1, KT // 2)]
        w0a = wpool.tile([128, 512], f16, tag="w0a", name="w0a")   # k=0 nh=0
        w0b = wpool.tile([128, 512], f16, tag="w0b", name="w0b")   # k=0 nh=1
        wts = [wpool.tile([128, N], f16, tag=f"w{k}", name=f"w{k}")
               for k in range(1, KT)]

        def xsrc(k, bt):
            """lhsT access for contraction chunk k, batch tile bt."""
            if k == 0:
                return x0a[:, 128 * bt:128 * (bt + 1)]
            if k == 1:
                return x0b[:, 128 * bt:128 * (bt + 1)]
            c = k // 2 - 1
            off = (k % 2) * BSH
            return xts[c][:, off + 128 * bt:off + 128 * (bt + 1)]

        def wsrc(k, nh):
            """rhs (moving) access for chunk k, column half nh."""
            if k == 0:
                return (w0a if nh == 0 else w0b)[:]
            return wts[k - 1][:, 512 * nh:512 * (nh + 1)]

        # issue order = chunk consumption order, alternating HWDGE queues
        # (sync/scalar) so both descriptor rings share the load and chunk k
        # lands roughly in order
        issue = [
            ("x", x0a, (0, 0, 1)),            # X2 row block 0, first half
            ("w0", w0a, (0, 512)),
            ("x", x0b, (0, 1, 2)),            # X2 row block 0, second half
            ("w0", w0b, (512, 1024)),
            ("w", wts[0], 1),
            ("w", wts[1], 2),
            ("x", xts[0], (1, 0, 2)),         # k=2,3
            ("w", wts[2], 3),
            ("w", wts[3], 4),
            ("x", xts[1], (2, 0, 2)),         # k=4,5
            ("w", wts[4], 5),
            ("w", wts[5], 6),
            ("x", xts[2], (3, 0, 2)),         # k=6,7
            ("w", wts[6], 7),
        ]
        for j, (kind, t, meta) in enumerate(issue):
            eng = nc.sync if j % 2 == 0 else nc.scalar
            if kind == "x":
                c, h0, h1 = meta
                eng.dma_start(
                    out=t[:],
                    in_=X2[128 * c:128 * (c + 1), h0 * BSH:h1 * BSH])
            elif kind == "w0":
                c0, c1 = meta
                eng.dma_start(out=t[:], in_=W[0:128, c0:c1])
            else:
                k = meta
                eng.dma_start(out=t[:], in_=W[128 * k:128 * (k + 1), :])

        # PE p-state warmup: the PE clock ramps 0.65 -> 1.2 -> 2.4 GHz only
        # after ~3us of continuous execution; burn part of the ramp on dummy
        # matmuls over a memset tile while the first chunks stream in.
        wa = opool.tile([128, 512], f16, name="warm")
        nc.gpsimd.memset(wa[:], 0.0)
        ps = ppool.tile([128, BT * NH * 512], f32, name="ps")
        for i in range(N_WARMUP):
            nc.tensor.matmul(
                ps[:, 0:512], wa[:, 0:128], wa[:],
                start=True, stop=True, skip_group_check=True,
            )

        # bank (bt, nh) holds out[128*bt:128*(bt+1), 512*nh:512*(nh+1)].
        # k-outer order keeps the PE gapless (matmul order == chunk arrival
        # order, so it never waits on a chunk that is later than its pace);
        # the final two k-layers run per-bank so the 8 bank stops stagger
        # ~2 matmul slots apart and evac/store pipeline into the PE tail.
        def bank(bt, nh):
            return ps[:, (bt * NH + nh) * 512:(bt * NH + nh + 1) * 512]

        banks = [(bt, nh) for bt in range(BT) for nh in range(NH)]
        for k in range(KT - 2):
            for bt, nh in banks:
                nc.tensor.matmul(
                    bank(bt, nh), xsrc(k, bt), wsrc(k, nh),
                    start=(k == 0), stop=False,
                )
        ots = [opool.tile([128, N], f16, name=f"o{bt}") for bt in range(BT)]
        for i, (bt, nh) in enumerate(banks):
            nc.tensor.matmul(bank(bt, nh), xsrc(KT - 2, bt), wsrc(KT - 2, nh),
                             start=False, stop=False)
            nc.tensor.matmul(bank(bt, nh), xsrc(KT - 1, bt), wsrc(KT - 1, nh),
                             start=False, stop=True)
            # evacuate as soon as this bank stops; DVE/ACT alternate so the
            # copy pipeline (~0.69us each) keeps up with the ~0.43us stagger
            ot = ots[bt]
            ceng = nc.vector.tensor_copy if i % 2 == 0 else nc.scalar.copy
            ceng(ot[:, 512 * nh:512 * (nh + 1)], bank(bt, nh))
            deng = nc.sync if i % 2 == 0 else nc.scalar
            deng.dma_start(
                out=OUT[128 * bt:128 * (bt + 1), 512 * nh:512 * (nh + 1)],
                in_=ot[:, 512 * nh:512 * (nh + 1)])

    nc.compile()
    return nc


def _get_nc():
    if "nc" not in _CACHE:
        _CACHE["nc"] = _build_nc()
    return _CACHE["nc"]


def _in_maps(x, T):
    """Per-core input maps: core = bg * 2 + cg (bg batch group, cg re|im)."""
    xT = x.T.astype(np.float16)                            # [N, B]
    Wre = np.ascontiguousarray(T.real.T.astype(np.float16))  # [j, n]
    Wim = np.ascontiguousarray(T.imag.T.astype(np.float16))
    maps = []
    for core in range(8):
        bg, cg = divmod(core, C_GROUPS)
        xs = xT[:, bg * BSH:(bg + 1) * BSH]                # [N, BSH]
        # X2[kc*128 + p, i*512 + b] = xs[kc*256 + i*128 + p, b]
        x2 = np.ascontiguousarray(
            xs.reshape(N // 256, 2, 128, BSH)
            .transpose(0, 2, 1, 3)
            .reshape(N // 2, 2 * BSH))
        maps.append({
            "X2": x2,
            "W": Wre if cg == 0 else Wim,
        })
    return maps


def _assemble(results):
    out = np.empty((B, N), dtype=np.complex64)
    for core in range(8):
        bg, cg = divmod(core, C_GROUPS)
        o = results[core]["OUT"].astype(np.float32)        # [BSH, N]
        if cg == 0:
            out.real[bg * BSH:(bg + 1) * BSH, :] = o
        else:
            out.imag[bg * BSH:(bg + 1) * BSH, :] = o
    return out


def kernel(x, params, split, atten, index):
    from concourse.bass_utils import run_bass_kernel_spmd

    x = np.asarray(x, dtype=np.float32)
    T = _fold(np.asarray(params), np.asarray(split), np.asarray(atten),
              np.asarray(index))
    nc = _get_nc()
    res = run_bass_kernel_spmd(nc, _in_maps(x, T), list(range(8)))
    return _assemble(res.results)


# revision 10
# speedup vs baseline: 1.1966x; 1.0880x over previous
"""Trainium2 kernel for nn_ClementsPSBS (Clements photonic mesh, 1024 layers).

Strategy: the whole network is linear in x (complex transfer matrix), so we
fold all 1024 layers of 2x2 rotations + attenuation into a single complex
matrix T (host-side, cheap), then the HW kernel is out = x @ T^T computed as
two real matmuls distributed over 8 NeuronCores:
  - 4 batch groups (512 rows each) x 2 column groups (real part | imag part)
  - per core: OUT[512b, 1024n] = xT[1024k, 512b]^T @ W[1024k, 1024n]
    with x-chunks stationary in the PE and W-chunks moving, fp16 in/out.
DMA: x/W chunk streams alternate between the two HWDGE queues (sync+scalar)
so input bandwidth is not bottlenecked on one descriptor ring; outputs are
evacuated per-PSUM-bank as soon as each bank's accumulation stops, so the
store overlaps the matmul tail.
"""

import numpy as np

N = 1024          # features
L = 1024          # layers
B = 2048          # batch
NA = N // 2       # pairs per layer
R_GROUPS = 4      # batch groups across cores
C_GROUPS = 2      # column groups (re | im)
BSH = B // R_GROUPS  # 512 batch rows per core

KT = N // 128     # 8 contraction chunks
BT = BSH // 128   # 4 batch tiles (PE stationary dim)
NH = N // 512     # 2 column halves (PSUM bank width)

_CACHE = {}


# ---------------------------------------------------------------------------
# Host-side fold: collapse 1024 layers into one complex transfer matrix T
# such that out = x @ T.T  (T[n, j]: coefficient of input feature j in
# output feature n).
# ---------------------------------------------------------------------------

def _expected_index():
    nA = N // 2
    iA = np.array([[2 * i, 2 * i + 1] for i in range(nA)], dtype=np.int32)
    iB = np.array([[2 * i + 1, 2 * i + 2] for i in range(nA - 1)]
                  + [[~0, ~(N - 1)]], dtype=np.int32)
    layers = [iA if l % 2 == 0 else iB for l in range(L)]
    return np.stack(layers).astype(np.int32)


def _coeffs(params, split, atten, index):
    """Per-layer per-pair 2x2 complex coefficients with attenuation folded in.

    Layer update for pair (p, q):
      u[p]' = at[p]*(cos(a)*e^{i th} * u[p] + i sin(a) * u[q])
      u[q]' = at[q]*(i sin(a)*e^{i th} * u[p] + cos(a) * u[q])
    Rows untouched by a pair still get u *= at.
    """
    theta = params[0].astype(np.float64)          # [L, NA]
    alpha = np.pi / 4 + split.astype(np.float64)  # [L, NA]
    eith = np.exp(1j * theta)
    c = np.cos(alpha)
    s = 1j * np.sin(alpha)
    A = c * eith
    Bc = s + 0j * s
    Cc = s * eith
    D = c + 0j * c
    return A, Bc, Cc, D


def _fold_fast(params, split, atten, index):
    """jax-CPU scan fold for the standard even/odd Clements pattern."""
    import jax
    import jax.numpy as jnp

    A, Bc, Cc, D = _coeffs(params, split, atten, index)
    at = atten.astype(np.complex128)              # [L, N]

    # even layers: pairs (2i, 2i+1), all N rows rotated
    ev = slice(0, L, 2)
    at_p_e = at[ev][:, 0::2]                      # [L/2, NA]
    at_q_e = at[ev][:, 1::2]
    Ae = (A[ev] * at_p_e).astype(np.complex64)
    Be = (Bc[ev] * at_p_e).astype(np.complex64)
    Ce = (Cc[ev] * at_q_e).astype(np.complex64)
    De = (D[ev] * at_q_e).astype(np.complex64)

    # odd layers: pairs (2i+1, 2i+2) for i < NA-1; rows 0 and N-1 only atten
    od = slice(1, L, 2)
    at_p_o = at[od][:, 1:N - 1:2]                 # [L/2, NA-1]
    at_q_o = at[od][:, 2:N:2]
    Ao = (A[od][:, :NA - 1] * at_p_o).astype(np.complex64)
    Bo = (Bc[od][:, :NA - 1] * at_p_o).astype(np.complex64)
    Co = (Cc[od][:, :NA - 1] * at_q_o).astype(np.complex64)
    Do = (D[od][:, :NA - 1] * at_q_o).astype(np.complex64)
    at0 = at[od][:, 0].astype(np.complex64)       # [L/2]
    atN = at[od][:, N - 1].astype(np.complex64)

    cpu = jax.devices('cpu')[0]

    def step(T, co):
        ae, be, ce, de, ao, bo, co_, do, a0, aN = co
        Tr = T.reshape(NA, 2, N)
        p = Tr[:, 0, :]
        q = Tr[:, 1, :]
        np_ = ae[:, None] * p + be[:, None] * q
        nq = ce[:, None] * p + de[:, None] * q
        T = jnp.stack([np_, nq], axis=1).reshape(N, N)
        mid = T[1:N - 1].reshape(NA - 1, 2, N)
        p = mid[:, 0, :]
        q = mid[:, 1, :]
        np_ = ao[:, None] * p + bo[:, None] * q
        nq = co_[:, None] * p + do[:, None] * q
        midn = jnp.stack([np_, nq], axis=1).reshape(N - 2, N)
        T = jnp.concatenate([T[0:1] * a0, midn, T[N - 1:] * aN], axis=0)
        return T, None

    with jax.default_device(cpu):
        T0 = jnp.eye(N, dtype=jnp.complex64)
        coeffs = (Ae, Be, Ce, De, Ao, Bo, Co, Do, at0, atN)
        coeffs = jax.tree.map(jnp.asarray, coeffs)
        fold = jax.jit(lambda T0, co: jax.lax.scan(step, T0, co)[0])
        T = fold(T0, coeffs)
        return np.asarray(T)


def _fold_general(params, split, atten, index):
    """Reference-faithful fold for arbitrary index content (numpy)."""
    A, Bc, Cc, D = _coeffs(params, split, atten, index)
    T = np.eye(N, dtype=np.complex128)
    at = atten.astype(np.complex128)
    for l in range(L):
        idx = index[l]
        valid = (idx >= 0).all(axis=1)
        gi = np.mod(idx, N)
        p = gi[valid, 0]
        q = gi[valid, 1]
        Tp = T[p, :].copy()
        Tq = T[q, :].copy()
        T[p, :] = A[l][valid][:, None] * Tp + Bc[l][valid][:, None] * Tq
        T[q, :] = Cc[l][valid][:, None] * Tp + D[l][valid][:, None] * Tq
        T *= at[l][:, None]
    return T.astype(np.complex64)


def _fold(params, split, atten, index):
    if np.array_equal(index, _expected_index()):
        try:
            return _fold_fast(params, split, atten, index)
        except Exception:
            pass
    return _fold_general(params, split, atten, index)


# ---------------------------------------------------------------------------
# Device kernel: OUT[512b, 1024n] = xT[1024k, 512b]^T @ W[1024k, 1024n]
# ---------------------------------------------------------------------------

N_WARMUP = 5      # PE p-state warmup matmuls before real data arrives


def _build_nc():
    import concourse.bass as bass
    import concourse.bacc as bacc
    import concourse.mybir as mybir
    import concourse.tile as tile
    from contextlib import ExitStack

    f32 = mybir.dt.float32
    f16 = mybir.dt.float16

    nc = bacc.Bacc("TRN2", target_bir_lowering=False, debug=False,
                   num_devices=8)
    X = nc.dram_tensor("X", [N, BSH], f16, kind="ExternalInput").ap()
    W = nc.dram_tensor("W", [N, N], f16, kind="ExternalInput").ap()
    OUT = nc.dram_tensor("OUT", [BSH, N], f16, kind="ExternalOutput").ap()

    with tile.TileContext(nc) as tc, ExitStack() as ctx:
        xpool = ctx.enter_context(tc.tile_pool(name="xp", bufs=1))
        wpool = ctx.enter_context(tc.tile_pool(name="wp", bufs=1))
        opool = ctx.enter_context(tc.tile_pool(name="op", bufs=1))
        ppool = ctx.enter_context(tc.tile_pool(name="pp", bufs=1, space="PSUM"))

        # Input tiles: per chunk k, three ~128KB pieces (x_k, W_k nh0 half,
        # W_k nh1 half) spread over both HWDGE queues so every chunk's
        # pieces finish together and strictly in chunk order on each queue.
        xts = [xpool.tile([128, BSH], f16, tag=f"x{k}", name=f"x{k}")
               for k in range(KT)]
        wts = [[wpool.tile([128, 512], f16, tag=f"w{k}_{nh}",
                           name=f"w{k}_{nh}") for nh in range(NH)]
               for k in range(KT)]

        def xsrc(k, bt):
            return xts[k][:, 128 * bt:128 * (bt + 1)]

        def wsrc(k, nh):
            return wts[k][nh][:]

        for k in range(KT):
            ex = nc.sync if k % 2 == 0 else nc.scalar
            ew = nc.scalar if k % 2 == 0 else nc.sync
            ex.dma_start(out=xts[k][:], in_=X[128 * k:128 * (k + 1), :])
            ew.dma_start(out=wts[k][0][:],
                         in_=W[128 * k:128 * (k + 1), 0:512])
            ex.dma_start(out=wts[k][1][:],
                         in_=W[128 * k:128 * (k + 1), 512:1024])

        # PE p-state warmup: the PE clock ramps 0.65 -> 1.2 -> 2.4 GHz only
        # after ~3us of continuous execution; burn part of the ramp on dummy
        # matmuls over a memset tile while the first chunks stream in.
        wa = opool.tile([128, 512], f16, name="warm")
        nc.gpsimd.memset(wa[:], 0.0)
        ps = ppool.tile([128, BT * NH * 512], f32, name="ps")
        for i in range(N_WARMUP):
            nc.tensor.matmul(
                ps[:, 0:512], wa[:, 0:128], wa[:],
                start=True, stop=True, skip_group_check=True,
            )

        # bank (bt, nh) holds out[128*bt:128*(bt+1), 512*nh:512*(nh+1)].
        # k-outer order keeps the PE gapless (matmul order == chunk arrival
        # order, so it never waits on a chunk that is later than its pace);
        # the final two k-layers run per-bank so the 8 bank stops stagger
        # ~2 matmul slots apart and evac/store pipeline into the PE tail.
        def bank(bt, nh):
            return ps[:, (bt * NH + nh) * 512:(bt * NH + nh + 1) * 512]

        banks = [(bt, nh) for bt in range(BT) for nh in range(NH)]
        for k in range(KT - 2):
            for bt, nh in banks:
                nc.tensor.matmul(
                    bank(bt, nh), xsrc(k, bt), wsrc(k, nh),
                    start=(k == 0), stop=False,
                )
        ots = [opool.tile([128, N], f16, name=f"o{bt}") for bt in range(BT)]
        for i, (bt, nh) in enumerate(banks):
            nc.tensor.matmul(bank(bt, nh), xsrc(KT - 2, bt), wsrc(KT - 2, nh),
                             start=False, stop=False)
            nc.tensor.matmul(bank(bt, nh), xsrc(KT - 1, bt), wsrc(KT - 1, nh),
                             start=False, stop=True)
            # evacuate as soon as this bank stops; DVE/ACT alternate so the
            # copy pipeline (~0.69us each) keeps up with the ~0.43us stagger
            ot = ots[bt]
            ceng = nc.vector.tensor_copy if i % 2 == 0 else nc.scalar.copy
            ceng(ot[:, 512 * nh:512 * (nh + 1)], bank(bt, nh))
            deng = nc.sync if i % 2 == 0 else nc.scalar
            deng.dma_start(
                out=OUT[128 * bt:128 * (bt + 1), 512 * nh:512 * (nh + 1)],
                in_=ot[:, 512 * nh:512 * (nh + 1)])

    nc.compile()
    return nc


def _get_nc():
    if "nc" not in _CACHE:
        _CACHE["nc"] = _build_nc()
    return _CACHE["nc"]


def _in_maps(x, T):
    """Per-core input maps: core = bg * 2 + cg (bg batch group, cg re|im)."""
    xT = x.T.astype(np.float16)                            # [N, B]
    Wre = np.ascontiguousarray(T.real.T.astype(np.float16))  # [j, n]
    Wim = np.ascontiguousarray(T.imag.T.astype(np.float16))
    maps = []
    for core in range(8):
        bg, cg = divmod(core, C_GROUPS)
        xs = xT[:, bg * BSH:(bg + 1) * BSH]                # [N, BSH]
        maps.append({
            "X": np.ascontiguousarray(xs),
            "W": Wre if cg == 0 else Wim,
        })
    return maps


def _assemble(results):
    out = np.empty((B, N), dtype=np.complex64)
    for core in range(8):
        bg, cg = divmod(core, C_GROUPS)
        o = results[core]["OUT"].astype(np.float32)        # [BSH, N]
        if cg == 0:
            out.real[bg * BSH:(bg + 1) * BSH, :] = o
        else:
            out.imag[bg * BSH:(bg + 1) * BSH, :] = o
    return out


def kernel(x, params, split, atten, index):
    from concourse.bass_utils import run_bass_kernel_spmd

    x = np.asarray(x, dtype=np.float32)
    T = _fold(np.asarray(params), np.asarray(split), np.asarray(atten),
              np.asarray(index))
    nc = _get_nc()
    res = run_bass_kernel_spmd(nc, _in_maps(x, T), list(range(8)))
    return _assemble(res.results)
